# revision 7
# baseline (speedup 1.0000x reference)
"""Cost-volume kernel for Trainium2 (Bass/Tile), SPMD over 8 NeuronCores.

volume[b, d, h, w] = mean_c left[b,c,h,w] * right[b,c,h,w-d],  0 for w < d.

Per core (one batch image b), per 8-row chunk:
  - Host pre-reverses right along w (rp[c,x] = R[c, 319-x]) and pre-scales
    left by 1/64 (the channel mean, exact power of two).  rp gets a 48-col
    zero margin at x in [320,368) => exact zeros for w < d.
  - TensorE (bf16): per (hh, wb) a [64,112] matmul
      G[q, f] = sum_c L[c, 64*wb+q] * rp[c, 256-64*wb+f]
    i.e. G[q, f] = vol_unscaled[d = q + f - 63, h, w = 64*wb + q].
    The two hh-halves (hh, hh+4) pack into rows 0:64 / 64:128 of shared
    [128, 448] PSUM tiles, 4 regions per tile, 5 tiles per chunk.
  - DVE/ACT evict PSUM -> SBUF band [128, 2240] (f32 -> bf16 cast).
  - ONE contiguous store DMA per chunk: band -> DRAM out[c] (4480B runs,
    full DMA bandwidth; no on-chip skew at all).
Host: gather the 48 diagonals f = 63 - q + d from the band (pure selection,
numpy take_along_axis), reshape to [D, H, W], upcast to f32.

Cost model (matches TimelineSim): DMA_ENGINES busy/chunk = 1820ns loads +
1593ns band store; 3 DMAs/chunk keeps the shared HWDGE (~630ns/DMA) well
under the DMA transfer time.  The raw band (112 cols per 64-wide w-block)
stores 2.33x the output bytes but at full 360GB/s; a diagonal-extracting
store would need a sub-row partition step on the DMA *read* side, which the
BIR verifier rejects ("illegal partition step" - legal on writes only).
"""

import sys

sys.path.insert(0, "/opt/trn_rl_repo")

import numpy as np

import concourse.bass as bass
import concourse.tile as tile
from concourse import bacc, mybir
from concourse.ap import AP

B, C, H, W, D = 8, 64, 160, 320, 48
MARGIN = 48
RPW = W + MARGIN             # 368
BM = 64                      # w-block size
BANDW = BM + MARGIN          # 112 band cols per region
NWB = W // BM                # 5 blocks
CH = 8                       # h rows per chunk
NKP = 4 * NWB                # 20 packed regions per chunk (4 hh x 5 wb)
PKW = NKP * BANDW            # 2240 band cols
NTILE = 5                    # PSUM tiles per chunk (4 regions each)

_cache = {}


def _build(h_count=H, reps=1):
    bf16 = mybir.dt.bfloat16
    f32 = mybir.dt.float32
    assert h_count % CH == 0
    nchunk = h_count // CH

    nc = bacc.Bacc("TRN2", target_bir_lowering=False, debug=False)
    left = nc.dram_tensor("left", [C, h_count, W], bf16, kind="ExternalInput")
    right = nc.dram_tensor("right", [C, h_count, W], bf16, kind="ExternalInput")
    if reps != 1:
        # unused; forces a distinct HLO per reps so the jit/NEFF caches
        # cannot alias timing builds of different rep counts
        nc.dram_tensor("rep_tag", [1, 8 * reps], mybir.dt.float32,
                       kind="ExternalInput")
    out = nc.dram_tensor("out", [nchunk, 128, PKW], bf16, kind="ExternalOutput")

    with tile.TileContext(nc) as tc:
        # persistent right-tiles: margins zeroed ONCE (not per chunk), so the
        # Pool queue stays free for the band stores and loads never queue
        # behind a memset
        rps = [
            nc.alloc_sbuf_tensor(f"rp{k}", [C, CH, RPW], bf16) for k in range(6)
        ]
        for rp in rps:
            nc.gpsimd.memset(rp[:, :, W:RPW].bitcast(f32), 0.0)
        with (
            tc.tile_pool(name="lt", bufs=6) as lt_pool,
            tc.tile_pool(name="ps", bufs=8, space="PSUM") as ps_pool,
            tc.tile_pool(name="band", bufs=3) as band_pool,
        ):
            for ci in range(reps * nchunk):
                c = ci % nchunk
                h0 = c * CH
                lt = lt_pool.tile([C, CH, W], bf16)
                nc.sync.dma_start(lt[:], left[:, h0 : h0 + CH, :])
                rp = rps[ci % 6]
                nc.sync.dma_start(rp[:, :, 0:W], right[:, h0 : h0 + CH, :])

                pts = [
                    ps_pool.tile([128, 4 * BANDW], f32, tag="ps", name=f"ps{t}")
                    for t in range(NTILE)
                ]
                for hh4 in range(4):
                    for wb in range(NWB):
                        kp = hh4 * NWB + wb
                        t, c0 = kp // 4, (kp % 4) * BANDW
                        for g in range(2):
                            hh = hh4 + 4 * g
                            nc.tensor.matmul(
                                pts[t][64 * g : 64 * g + 64, c0 : c0 + BANDW],
                                lt[:, hh, BM * wb : BM * wb + BM],
                                rp[:, hh, 4 * BM - BM * wb : 4 * BM - BM * wb + BANDW],
                                start=True,
                                stop=True,
                            )

                bb = band_pool.tile([128, PKW], bf16, tag="band")
                for t in range(NTILE):
                    dst = bb[:, 4 * BANDW * t : 4 * BANDW * (t + 1)]
                    if t % 2 == 0:
                        nc.vector.tensor_copy(dst, pts[t][:])
                    else:
                        nc.scalar.copy(dst, pts[t][:])

                # Pool/SWDGE: keeps the store off both the SP queue (would
                # serialize next chunk's loads behind this chunk's eviction
                # sem wait) and the shared HWDGE
                nc.gpsimd.dma_start(out[c], bb[:])

    nc.compile()
    return nc


def _get_nc():
    key = H
    if key not in _cache:
        _cache[key] = _build()
    return _cache[key]


def _prep(left_feature, right_feature):
    import ml_dtypes

    lf = np.asarray(left_feature, dtype=np.float32) * np.float32(1.0 / C)
    rf = np.asarray(right_feature, dtype=np.float32)[:, :, :, ::-1]
    lf = lf.astype(ml_dtypes.bfloat16)
    rf = np.ascontiguousarray(rf).astype(ml_dtypes.bfloat16)
    return lf, rf


# f_sel[q, d] = 63 - q + d : band col of diagonal d for in-block col q
_FSEL = (63 - np.arange(BM)[:, None] + np.arange(D)[None, :])[
    None, None, :, None, None, :
]


def _extract(band):
    """[nchunk, 128, PKW] bf16 band -> [D, H, W] f32 volume (pure selection)."""
    nchunk = band.shape[0]
    br = np.asarray(band, dtype=np.float32).reshape(nchunk, 2, BM, 4, NWB, BANDW)
    ext = np.take_along_axis(br, np.broadcast_to(
        _FSEL, (nchunk, 2, BM, 4, NWB, D)), axis=5)
    # [c, g, q, hh4, wb, d] -> [d, c, g, hh4, wb, q] -> [D, H, W]
    return np.ascontiguousarray(ext.transpose(5, 0, 1, 3, 4, 2)).reshape(D, H, W)


def kernel(left_feature, right_feature, disp):
    from concourse.bass_utils import run_bass_kernel_spmd

    assert int(disp) == D, f"kernel hardcoded for disp={D}, got {disp}"
    lf, rf = _prep(left_feature, right_feature)
    assert lf.shape == (B, C, H, W), lf.shape

    nc = _get_nc()
    in_maps = [{"left": lf[b], "right": rf[b]} for b in range(B)]
    res = run_bass_kernel_spmd(nc, in_maps, list(range(B)))

    vol = np.empty((B, D, H, W), dtype=np.float32)
    for b in range(B):
        vol[b] = _extract(np.asarray(res.results[b]["out"]))
    return vol


# revision 9
# speedup vs baseline: 16.1367x; 16.1367x over previous
"""Cost-volume kernel for Trainium2 (Bass/Tile), SPMD over 8 NeuronCores.

volume[b, d, h, w] = mean_c left[b,c,h,w] * right[b,c,h,w-d],  0 for w < d.

Per core (one batch image b), per 8-row chunk:
  - Host pre-reverses right along w (rp[c,x] = R[c, 319-x]) and pre-scales
    left by 1/64 (the channel mean, exact power of two).  rp gets a 48-col
    zero margin at x in [320,368) => exact zeros for w < d.
  - TensorE (bf16): per (hh, wb) a [32,80] matmul (w-blocks of 32)
      G[q, f] = sum_c L[c, 32*wb+q] * rp[c, 288-32*wb+f]
    i.e. G[q, f] = vol_unscaled[d = q + f - 31, h, w = 32*wb + q].
    Four hh's stack at 32-row offsets (legal matmul tile positions
    {0,32,64,96}) into [128, 320] PSUM tiles, 4 col-regions per tile,
    5 tiles per chunk.
  - DVE/ACT evict PSUM -> SBUF band [128, 1600] (f32 -> bf16 cast).
  - ONE contiguous store DMA per chunk: band -> DRAM out[c] (3200B runs,
    full DMA bandwidth; no on-chip skew at all).
Host: gather the 48 diagonals f = 31 - q + d from the band (pure selection,
numpy take_along_axis), reshape to [D, H, W], upcast to f32.

Cost model (matches TimelineSim): the 80-wide band regions store 1.67x the
output bytes but at full 360GB/s contiguous rate; DMA_ENGINES busy/chunk =
1820ns loads + 1138ns band store.  BM=32 balances store overhead (ratio
(BM+48)/48) against PE column time (ratio (BM+48)/BM): stores 22.8us,
PE 53.3us, loads 36.4us -> DMA-bound at ~59us busy.  A diagonal-extracting
store would need a sub-row partition step on the DMA *read* side, which the
BIR verifier rejects ("illegal partition step" - legal on writes only);
hence store-the-band + host selection.  Loads on SP/HWDGE, store on
Pool/SWDGE so no queue ever blocks the next chunk's loads behind an
eviction semaphore.
"""

import sys

sys.path.insert(0, "/opt/trn_rl_repo")

import numpy as np

import concourse.bass as bass
import concourse.tile as tile
from concourse import bacc, mybir
from concourse.ap import AP

B, C, H, W, D = 8, 64, 160, 320, 48
MARGIN = 48
RPW = W + MARGIN             # 368
BM = 32                      # w-block size
BANDW = BM + MARGIN          # 80 band cols per region
NWB = W // BM                # 10 blocks
CH = 8                       # h rows per chunk
NK = 2 * NWB                 # 20 col-regions per chunk (2 sections x 10 wb)
PKW = NK * BANDW             # 1600 band cols
NTILE = 5                    # PSUM tiles per chunk (4 regions each)

_cache = {}


def _build(h_count=H, reps=1):
    bf16 = mybir.dt.bfloat16
    f32 = mybir.dt.float32
    assert h_count % CH == 0
    nchunk = h_count // CH

    nc = bacc.Bacc("TRN2", target_bir_lowering=False, debug=False)
    left = nc.dram_tensor("left", [C, h_count, W], bf16, kind="ExternalInput")
    right = nc.dram_tensor("right", [C, h_count, W], bf16, kind="ExternalInput")
    if reps != 1:
        # unused; forces a distinct HLO per reps so the jit/NEFF caches
        # cannot alias timing builds of different rep counts
        nc.dram_tensor("rep_tag", [1, 8 * reps], mybir.dt.float32,
                       kind="ExternalInput")
    out = nc.dram_tensor("out", [nchunk, 128, PKW], bf16, kind="ExternalOutput")

    with tile.TileContext(nc) as tc:
        # persistent right-tiles: margins zeroed ONCE (not per chunk), so the
        # Pool queue stays free for the band stores and loads never queue
        # behind a memset
        rps = [
            nc.alloc_sbuf_tensor(f"rp{k}", [C, CH, RPW], bf16) for k in range(6)
        ]
        for rp in rps:
            nc.gpsimd.memset(rp[:, :, W:RPW].bitcast(f32), 0.0)
        with (
            tc.tile_pool(name="lt", bufs=6) as lt_pool,
            tc.tile_pool(name="ps", bufs=8, space="PSUM") as ps_pool,
            tc.tile_pool(name="band", bufs=3) as band_pool,
        ):
            for ci in range(reps * nchunk):
                c = ci % nchunk
                h0 = c * CH
                lt = lt_pool.tile([C, CH, W], bf16)
                nc.sync.dma_start(lt[:], left[:, h0 : h0 + CH, :])
                rp = rps[ci % 6]
                nc.sync.dma_start(rp[:, :, 0:W], right[:, h0 : h0 + CH, :])

                pts = [
                    ps_pool.tile([128, 4 * BANDW], f32, tag="ps", name=f"ps{t}")
                    for t in range(NTILE)
                ]
                for K in range(NK):
                    wb, sec = K % NWB, K // NWB
                    t, c0 = K // 4, (K % 4) * BANDW
                    x0 = (NWB - 1) * BM - BM * wb  # 288 - 32*wb
                    for s in range(4):
                        hh = 4 * sec + s
                        nc.tensor.matmul(
                            pts[t][32 * s : 32 * s + 32, c0 : c0 + BANDW],
                            lt[:, hh, BM * wb : BM * wb + BM],
                            rp[:, hh, x0 : x0 + BANDW],
                            start=True,
                            stop=True,
                            tile_position=(0, 32 * s),
                        )

                bb = band_pool.tile([128, PKW], bf16, tag="band")
                for t in range(NTILE):
                    dst = bb[:, 4 * BANDW * t : 4 * BANDW * (t + 1)]
                    if t % 2 == 0:
                        nc.vector.tensor_copy(dst, pts[t][:])
                    else:
                        nc.scalar.copy(dst, pts[t][:])

                # Pool/SWDGE: keeps the store off both the SP queue (would
                # serialize next chunk's loads behind this chunk's eviction
                # sem wait) and the shared HWDGE
                nc.gpsimd.dma_start(out[c], bb[:])

    nc.compile()
    return nc


def _get_nc():
    key = H
    if key not in _cache:
        _cache[key] = _build()
    return _cache[key]


def _prep(left_feature, right_feature):
    import ml_dtypes

    lf = np.asarray(left_feature, dtype=np.float32) * np.float32(1.0 / C)
    rf = np.asarray(right_feature, dtype=np.float32)[:, :, :, ::-1]
    lf = lf.astype(ml_dtypes.bfloat16)
    rf = np.ascontiguousarray(rf).astype(ml_dtypes.bfloat16)
    return lf, rf


# f_sel[q, d] = 31 - q + d : band col of diagonal d for in-block col q
_FSEL = (BM - 1 - np.arange(BM)[:, None] + np.arange(D)[None, :])[
    None, None, :, None, None, :
]


def _extract(band):
    """[nchunk, 128, PKW] bf16 band -> [D, H, W] f32 volume (pure selection)."""
    nchunk = band.shape[0]
    # [c, s, q, sec, wb, f']: p = 32*s + q, col = 80*(10*sec + wb) + f'
    br = np.asarray(band, dtype=np.float32).reshape(nchunk, 4, BM, 2, NWB, BANDW)
    ext = np.take_along_axis(br, np.broadcast_to(
        _FSEL, (nchunk, 4, BM, 2, NWB, D)), axis=5)
    # h = 8c + 4*sec + s, w = 32*wb + q, d
    # [c, s, q, sec, wb, d] -> [d, c, sec, s, wb, q] -> [D, H, W]
    return np.ascontiguousarray(ext.transpose(5, 0, 3, 1, 4, 2)).reshape(D, H, W)


def kernel(left_feature, right_feature, disp):
    from concourse.bass_utils import run_bass_kernel_spmd

    assert int(disp) == D, f"kernel hardcoded for disp={D}, got {disp}"
    lf, rf = _prep(left_feature, right_feature)
    assert lf.shape == (B, C, H, W), lf.shape

    nc = _get_nc()
    in_maps = [{"left": lf[b], "right": rf[b]} for b in range(B)]
    res = run_bass_kernel_spmd(nc, in_maps, list(range(B)))

    vol = np.empty((B, D, H, W), dtype=np.float32)
    for b in range(B):
        vol[b] = _extract(np.asarray(res.results[b]["out"]))
    return vol
